# revision 9
# baseline (speedup 1.0000x reference)
"""Differential attention kernel for 8 trn2 NeuronCores.

Sharding: (batch, head-group) over 8 cores. Core d handles batch b=d//4 and
heads [4*(d%4), 4*(d%4)+4). Each core:
  - projects q1,q2 in transposed pair layout qT [128, S] (2 heads per tile)
    and k1,k2 into per-head ZERO-PADDED tiles kz [128, S] (head rows in the
    matching parity half, other half zeroed) so every score matmul runs with
    K=128 -- fp32r matmuls at K=64 with alternating row groups are ~5x
    slower on TRN2, while K=128 runs at N cycles,
  - v in direct layout [S, 4, 65] with a ones column (denominator trick),
  - scores TRANSPOSED sT[j, i] (keys on partitions): mask bias becomes a
    per-partition ACT bias, PV matmul needs no transposes; exp via one
    [128, 2048] ACT op per key tile covering s1/s2 for both heads of a pair,
  - uT[65, 512] accumulated over key tiles; row 64 = softmax denominators,
  - combine o = u1/dn1 - lam*u2/dn2 via reciprocal_approx_fast + a DRAM
    round-trip partition-broadcast; lam folded into one scalar_tensor_tensor,
  - out-projection 4-way K=64 accumulation of Wo_hl.T @ o_hl -> partial
    outT [1024, S].
Host sums the 4 partial outT per batch (+bo) and transposes.
"""
import numpy as np

B, S, D, H = 2, 2048, 1024, 16
DH = D // H          # 64
SCALE = DH ** -0.5   # 0.125
NCORES = 8
HG = 4               # heads per device
KT = D // 128        # 8 contraction tiles over D
MT = D // 128        # 8 output tiles of qk projection (q1,q2,k1,k2 cols)
NCH = S // 512       # 4 query chunks
JT = S // 128        # 16 key tiles

_BUILD_CACHE = {}


def _build(lam: float):
    from contextlib import ExitStack
    import concourse.mybir as mybir
    import concourse.tile as tile
    from concourse import bacc

    f32 = mybir.dt.float32
    f32r = mybir.dt.float32r
    Exp = mybir.ActivationFunctionType.Exp
    mult = mybir.AluOpType.mult
    add = mybir.AluOpType.add

    nc = bacc.Bacc("TRN2", target_bir_lowering=False, debug=False,
                   num_devices=NCORES)

    xt_d = nc.dram_tensor("xt", [D, S], f32r, kind="ExternalInput").ap()
    wqk_d = nc.dram_tensor("wqk", [D, D], f32r, kind="ExternalInput").ap()
    wv_d = nc.dram_tensor("wv", [D, HG * DH], f32r, kind="ExternalInput").ap()
    wo_d = nc.dram_tensor("wo", [HG * DH, D], f32r, kind="ExternalInput").ap()
    bqk_d = nc.dram_tensor("bqk", [128, MT], f32, kind="ExternalInput").ap()
    bvc_d = nc.dram_tensor("bvc", [64, HG], f32, kind="ExternalInput").ap()
    maskb_d = nc.dram_tensor("maskb", [128, JT], f32,
                             kind="ExternalInput").ap()
    zpad_d = nc.dram_tensor("zpad", [64, S], f32r, kind="ExternalInput").ap()
    out_d = nc.dram_tensor("outT", [D, S], f32, kind="ExternalOutput").ap()

    with tile.TileContext(nc) as tc, ExitStack() as ctx:
        consts = ctx.enter_context(tc.tile_pool(name="consts", bufs=1))
        qk_pool = ctx.enter_context(tc.tile_pool(name="qk", bufs=1))
        v_pool = ctx.enter_context(tc.tile_pool(name="vp", bufs=1))
        ps = ctx.enter_context(tc.tile_pool(name="ps", bufs=1, space="PSUM"))

        bqk_sb = consts.tile([128, MT], f32)
        nc.sync.dma_start(out=bqk_sb, in_=bqk_d)
        bvc_sb = consts.tile([64, HG], f32)
        nc.sync.dma_start(out=bvc_sb, in_=bvc_d)
        maskb_sb = consts.tile([128, JT], f32)
        nc.sync.dma_start(out=maskb_sb, in_=maskb_d)
        # Wo rows grouped per local head hl
        wo_sb = [consts.tile([64, D], f32r, name=f"wo{i}", tag=f"wo{i}")
                 for i in range(HG)]
        for i in range(HG):
            nc.sync.dma_start(out=wo_sb[i], in_=wo_d[i * 64:(i + 1) * 64, :])
        ones1 = consts.tile([128, 1], f32)
        nc.vector.memset(ones1, 1.0)
        zeros1 = consts.tile([128, 1], f32)
        nc.vector.memset(zeros1, 0.0)

        # v in [S, HG, DH+1] layout; column DH holds ones (denominator trick)
        v_sb = v_pool.tile([128, JT, HG, DH + 1], f32r)
        nc.vector.tensor_copy(
            out=v_sb[:, :, :, DH:DH + 1],
            in_=ones1[:, None, None, :].broadcast_to([128, JT, HG, 1]))

        # q pair tiles: q_t[m][p], heads 2p (rows 0:64) and 2p+1 (rows 64:128)
        q_t = [[qk_pool.tile([128, S], f32r, name=f"q{m}p{p}",
                             tag=f"q{m}p{p}") for p in range(2)]
               for m in range(2)]
        # zero-padded k tiles: kz[m][hl] has k rows in parity half, 0 in other
        kz = [[qk_pool.tile([128, S], f32r, name=f"kz{m}h{hl}",
                            tag=f"kz{m}h{hl}") for hl in range(HG)]
              for m in range(2)]
        for m in range(2):
            for hl in range(HG):
                zh = 1 - (hl % 2)          # the half that must be zero
                zsl = slice(zh * 64, (zh + 1) * 64)
                nc.sync.dma_start(out=kz[m][hl][zsl, :], in_=zpad_d)

        # ---------------- projections ----------------
        projstack = ExitStack()
        projw = projstack.enter_context(tc.tile_pool(name="projw", bufs=1))
        projx = projstack.enter_context(tc.tile_pool(name="projx", bufs=1))

        wqk_sb = [projw.tile([128, D], f32r, name=f"wqk{k}", tag=f"wqk{k}")
                  for k in range(KT)]
        for k in range(KT):
            nc.sync.dma_start(out=wqk_sb[k],
                              in_=wqk_d[k * 128:(k + 1) * 128, :])
        wv_sb = [projw.tile([128, HG * DH], f32r, name=f"wv{k}", tag=f"wv{k}")
                 for k in range(KT)]
        for k in range(KT):
            nc.sync.dma_start(out=wv_sb[k],
                              in_=wv_d[k * 128:(k + 1) * 128, :])

        for nc_i in range(NCH):
            nsl = slice(nc_i * 512, (nc_i + 1) * 512)
            xtc = []
            for k in range(KT):
                x_one = projx.tile([128, 512], f32r, name="xtc", tag="xtc",
                                   bufs=8)
                nc.sync.dma_start(out=x_one,
                                  in_=xt_d[k * 128:(k + 1) * 128, nsl])
                xtc.append(x_one)
            # wqk col blocks: mt 0..3 = q1p0,q1p1,q2p0,q2p1; 4..7 = k1,k1,k2,k2
            for mt in (0, 2, 4, 6, 1, 3, 5, 7):
                pp = ps.tile([128, 512], f32, name="accp", tag="acc", bufs=4)
                for k in range(KT):
                    nc.tensor.matmul(
                        pp,
                        wqk_sb[k][:, mt * 128:(mt + 1) * 128],
                        xtc[k],
                        start=(k == 0), stop=(k == KT - 1))
                if mt < 4:
                    m, p = mt // 2, mt % 2
                    nc.vector.tensor_scalar_add(q_t[m][p][:, nsl], pp,
                                                bqk_sb[:, mt:mt + 1])
                else:
                    m, pr = (mt - 4) // 2, (mt - 4) % 2
                    for eps in range(2):
                        hl = 2 * pr + eps
                        esl = slice(eps * 64, (eps + 1) * 64)
                        nc.vector.tensor_scalar_add(
                            kz[m][hl][esl, nsl], pp[esl, :],
                            bqk_sb[esl, mt:mt + 1])
            # v projection for the 4 key tiles of this chunk
            for sl in range(4):
                st = nc_i * 4 + sl
                vp = ps.tile([128, HG * DH], f32, name="accv", tag="acc",
                             bufs=4)
                for k in range(KT):
                    nc.tensor.matmul(
                        vp,
                        xtc[k][:, sl * 128:(sl + 1) * 128],
                        wv_sb[k],
                        start=(k == 0), stop=(k == KT - 1))
                nc.vector.tensor_copy(
                    out=v_sb[:, st, :, 0:DH],
                    in_=vp.rearrange("p (h d) -> p h d", h=HG))

        projstack.close()

        # ---------------- attention ----------------
        e_pool = ctx.enter_context(tc.tile_pool(name="ep", bufs=2))
        oc_pool = ctx.enter_context(tc.tile_pool(name="oc", bufs=6))
        small = ctx.enter_context(tc.tile_pool(name="small", bufs=2))
        outst_pool = ctx.enter_context(tc.tile_pool(name="outst", bufs=3))
        scr_pool = ctx.enter_context(tc.tile_pool(name="scr", bufs=2,
                                                  space="DRAM"))

        for c in range(NCH):
            csl = slice(c * 512, (c + 1) * 512)
            ochl = [None] * HG
            for p in range(2):
                u_tiles = []
                for name in ("u1a", "u1b", "u2a", "u2b"):
                    u_tiles.append(ps.tile([DH + 1, 512], f32, name=name,
                                           tag="acc", bufs=4))
                for j in range(JT):
                    jsl = slice(j * 128, (j + 1) * 128)
                    s_ps = ps.tile([128, 2048], f32, name="s_ps", tag="s",
                                   bufs=1)
                    # sT[j, i] blocks: [s1h0 | s1h1 | s2h0 | s2h1]
                    for m in range(2):
                        for eps in range(2):
                            nc.tensor.matmul(
                                s_ps[:, (2 * m + eps) * 512:
                                        (2 * m + eps + 1) * 512],
                                kz[m][2 * p + eps][:, jsl],
                                q_t[m][p][:, csl],
                                start=True, stop=True)
                    e_sb = e_pool.tile([128, 2048], f32r, name="e_sb",
                                       tag="e")
                    nc.scalar.activation(e_sb, s_ps, Exp,
                                         bias=maskb_sb[:, j:j + 1],
                                         scale=SCALE)
                    # u accumulation; eps-outer so consecutive matmuls share
                    # the same stationary v tile
                    for eps in range(2):
                        for mi in range(2):
                            nc.tensor.matmul(
                                u_tiles[2 * mi + eps],
                                v_sb[:, j, 2 * p + eps, :],
                                e_sb[:, (2 * mi + eps) * 512:
                                        (2 * mi + eps + 1) * 512],
                                start=(j == 0), stop=(j == JT - 1))
                # combine: o_hl = u1/dn1 - lam*u2/dn2 (+ (1-lam)*bv)
                for eps in range(2):
                    hl = 2 * p + eps
                    u1 = u_tiles[0 + eps]
                    u2 = u_tiles[2 + eps]
                    u1_sb = small.tile([64, 512], f32, name="u1_sb",
                                       tag="u1_sb")
                    nc.vector.tensor_copy(out=u1_sb, in_=u1[0:DH, :])
                    u2_sb = small.tile([64, 512], f32, name="u2_sb",
                                       tag="u2_sb")
                    nc.vector.tensor_copy(out=u2_sb, in_=u2[0:DH, :])
                    rb = small.tile([DH + 1, 1024], f32, name="rb", tag="rb")
                    nc.vector.reciprocal(rb[DH:DH + 1, 0:512],
                                         u1[DH:DH + 1, :])
                    nc.vector.reciprocal(rb[DH:DH + 1, 512:1024],
                                         u2[DH:DH + 1, :])
                    # partition-broadcast r via a DRAM round-trip
                    scr = scr_pool.tile([1, 1024], f32, name="scr", tag="scr")
                    nc.sync.dma_start(out=scr, in_=rb[DH:DH + 1, :])
                    bc = small.tile([64, 1024], f32, name="bc", tag="bc")
                    nc.gpsimd.dma_start(
                        out=bc, in_=scr.partition_broadcast(64)[:, 0, :])
                    t1 = small.tile([64, 512], f32, name="t1", tag="t1")
                    nc.vector.tensor_tensor(t1, u1_sb, bc[:, 0:512], mult)
                    t2 = small.tile([64, 512], f32, name="t2", tag="t2")
                    nc.vector.tensor_tensor(t2, u2_sb, bc[:, 512:1024], mult)
                    oc_t = oc_pool.tile([64, 512], f32r, name="oc_t",
                                        tag="oc")
                    # oc = t1 - lam*t2 ; bv correction folded in by host when
                    # bv != 0 via bvc (adds one op)
                    nc.vector.scalar_tensor_tensor(
                        out=oc_t, in0=t2, scalar=-float(lam), in1=t1,
                        op0=mult, op1=add)
                    ochl[hl] = oc_t

            # out projection for this query chunk (K=64 per local head)
            for mt in range(MT):
                op = ps.tile([128, 512], f32, name="accop", tag="acc", bufs=4)
                for hl in range(HG):
                    nc.tensor.matmul(op,
                                     wo_sb[hl][:, mt * 128:(mt + 1) * 128],
                                     ochl[hl],
                                     start=(hl == 0), stop=(hl == HG - 1))
                outst = outst_pool.tile([128, 512], f32, name="outst",
                                        tag="outst")
                nc.vector.tensor_copy(out=outst, in_=op)
                nc.sync.dma_start(out=out_d[mt * 128:(mt + 1) * 128, csl],
                                  in_=outst)

    nc.compile()
    return nc


def _get_nc(lam: float):
    key = round(float(lam), 8)
    if key not in _BUILD_CACHE:
        _BUILD_CACHE[key] = _build(float(lam))
    return _BUILD_CACHE[key]


def _prep_in_maps(hidden_states, attention_mask, Wq, bq, Wk, bk, Wv, bv, Wo,
                  lam_f):
    in_maps = []
    for d in range(NCORES):
        b, g = d // 4, d % 4
        gc = slice(g * HG * DH, (g + 1) * HG * DH)   # 256 head-group columns
        xt = np.ascontiguousarray(hidden_states[b].T)
        wqk = np.ascontiguousarray(
            np.concatenate([Wq[:, :D][:, gc], Wq[:, D:][:, gc],
                            Wk[:, :D][:, gc], Wk[:, D:][:, gc]], axis=1))
        wv = np.ascontiguousarray(Wv[:, gc])
        wo = np.ascontiguousarray(Wo[gc, :])
        bqk = np.ascontiguousarray(
            np.concatenate([bq[:D][gc], bq[D:][gc], bk[:D][gc], bk[D:][gc]])
            .reshape(MT, 128).T)
        bvc = np.ascontiguousarray(
            ((1.0 - lam_f) * bv[gc]).reshape(HG, 64).T)
        maskb = np.ascontiguousarray(
            ((1.0 - attention_mask[b]) * -10000.0).reshape(JT, 128).T)
        in_maps.append({"xt": xt, "wqk": wqk, "wv": wv, "wo": wo,
                        "bqk": bqk, "bvc": bvc, "maskb": maskb,
                        "zpad": np.zeros((64, S), np.float32)})
    return in_maps


def kernel(hidden_states, attention_mask, Wq, bq, Wk, bk, Wv, bv, Wo, bo,
           lam):
    hidden_states = np.asarray(hidden_states, dtype=np.float32)
    attention_mask = np.asarray(attention_mask, dtype=np.float32)
    Wq = np.asarray(Wq, dtype=np.float32)
    bq = np.asarray(bq, dtype=np.float32)
    Wk = np.asarray(Wk, dtype=np.float32)
    bk = np.asarray(bk, dtype=np.float32)
    Wv = np.asarray(Wv, dtype=np.float32)
    bv = np.asarray(bv, dtype=np.float32)
    Wo = np.asarray(Wo, dtype=np.float32)
    bo = np.asarray(bo, dtype=np.float32)
    lam_f = float(np.asarray(lam))

    from concourse.bass_utils import run_bass_kernel_spmd

    nc = _get_nc(lam_f)
    in_maps = _prep_in_maps(hidden_states, attention_mask, Wq, bq, Wk, bk,
                            Wv, bv, Wo, lam_f)
    res = run_bass_kernel_spmd(nc, in_maps, core_ids=list(range(NCORES)))

    out = np.zeros((B, S, D), np.float32)
    for d in range(NCORES):
        out[d // 4] += res.results[d]["outT"].T
    out += bo
    # v-bias correction is linear: o += (1-lam)*bv @ Wo (exact; bv is zero in
    # the reference setup, so this is a no-op there)
    if np.any(bv != 0.0):
        out += ((1.0 - lam_f) * bv) @ Wo
    return out


# revision 10
# speedup vs baseline: 1.5300x; 1.5300x over previous
"""Differential attention kernel for 8 trn2 NeuronCores.

Sharding: (batch, head-group) over 8 cores. Core d handles batch b=d//4 and
heads [4*(d%4), 4*(d%4)+4). Each core:
  - projects q1,q2 in transposed pair layout qT [128, S] (2 heads per tile)
    and k1,k2 into per-head ZERO-PADDED tiles kz [128, S] (head rows in the
    matching parity half, other half zeroed) so every score matmul runs with
    K=128 -- fp32r matmuls at K=64 with alternating row groups are ~5x
    slower on TRN2, while K=128 runs at N cycles,
  - v in direct layout [S, 4, 65] with a ones column (denominator trick),
  - scores TRANSPOSED sT[j, i] (keys on partitions): mask bias becomes a
    per-partition ACT bias, PV matmul needs no transposes; exp via one
    [128, 2048] ACT op per key tile covering s1/s2 for both heads of a pair,
  - uT[65, 512] accumulated over key tiles; row 64 = softmax denominators,
  - combine o = u1/dn1 - lam*u2/dn2 via reciprocal_approx_fast + a DRAM
    round-trip partition-broadcast; lam folded into one scalar_tensor_tensor,
  - out-projection 4-way K=64 accumulation of Wo_hl.T @ o_hl -> partial
    outT [1024, S].
Host sums the 4 partial outT per batch (+bo) and transposes.
"""
import numpy as np

B, S, D, H = 2, 2048, 1024, 16
DH = D // H          # 64
SCALE = DH ** -0.5   # 0.125
NCORES = 8
HG = 4               # heads per device
KT = D // 128        # 8 contraction tiles over D
MT = D // 128        # 8 output tiles of qk projection (q1,q2,k1,k2 cols)
NCH = S // 512       # 4 query chunks
JT = S // 128        # 16 key tiles

_BUILD_CACHE = {}


def _build(lam: float):
    from contextlib import ExitStack
    import concourse.mybir as mybir
    import concourse.tile as tile
    from concourse import bacc

    f32 = mybir.dt.float32
    f32r = mybir.dt.float32r
    Exp = mybir.ActivationFunctionType.Exp
    mult = mybir.AluOpType.mult
    add = mybir.AluOpType.add

    nc = bacc.Bacc("TRN2", target_bir_lowering=False, debug=False,
                   num_devices=NCORES)

    xt_d = nc.dram_tensor("xt", [D, S], f32r, kind="ExternalInput").ap()
    wqk_d = nc.dram_tensor("wqk", [D, D], f32r, kind="ExternalInput").ap()
    wv_d = nc.dram_tensor("wv", [D, HG * DH], f32r, kind="ExternalInput").ap()
    wo_d = nc.dram_tensor("wo", [HG * DH, D], f32r, kind="ExternalInput").ap()
    bqk_d = nc.dram_tensor("bqk", [128, MT], f32, kind="ExternalInput").ap()
    bvc_d = nc.dram_tensor("bvc", [64, HG], f32, kind="ExternalInput").ap()
    maskb_d = nc.dram_tensor("maskb", [128, JT], f32,
                             kind="ExternalInput").ap()
    zpad_d = nc.dram_tensor("zpad", [64, S], f32r, kind="ExternalInput").ap()
    out_d = nc.dram_tensor("outT", [D, S], f32, kind="ExternalOutput").ap()

    with tile.TileContext(nc) as tc, ExitStack() as ctx:
        consts = ctx.enter_context(tc.tile_pool(name="consts", bufs=1))
        qk_pool = ctx.enter_context(tc.tile_pool(name="qk", bufs=1))
        v_pool = ctx.enter_context(tc.tile_pool(name="vp", bufs=1))
        ps = ctx.enter_context(tc.tile_pool(name="ps", bufs=1, space="PSUM"))

        bqk_sb = consts.tile([128, MT], f32)
        nc.sync.dma_start(out=bqk_sb, in_=bqk_d)
        bvc_sb = consts.tile([64, HG], f32)
        nc.sync.dma_start(out=bvc_sb, in_=bvc_d)
        maskb_sb = consts.tile([128, JT], f32)
        nc.sync.dma_start(out=maskb_sb, in_=maskb_d)
        # Wo rows grouped per local head hl
        wo_sb = [consts.tile([64, D], f32r, name=f"wo{i}", tag=f"wo{i}")
                 for i in range(HG)]
        for i in range(HG):
            nc.sync.dma_start(out=wo_sb[i], in_=wo_d[i * 64:(i + 1) * 64, :])
        ones1 = consts.tile([128, 1], f32)
        nc.vector.memset(ones1, 1.0)
        zeros1 = consts.tile([128, 1], f32)
        nc.vector.memset(zeros1, 0.0)

        # v in [S, HG, DH+1] layout; column DH holds ones (denominator trick)
        v_sb = v_pool.tile([128, JT, HG, DH + 1], f32r)
        nc.vector.tensor_copy(
            out=v_sb[:, :, :, DH:DH + 1],
            in_=ones1[:, None, None, :].broadcast_to([128, JT, HG, 1]))

        # q pair tiles: q_t[m][p], heads 2p (rows 0:64) and 2p+1 (rows 64:128)
        q_t = [[qk_pool.tile([128, S], f32r, name=f"q{m}p{p}",
                             tag=f"q{m}p{p}") for p in range(2)]
               for m in range(2)]
        # zero-padded k tiles: kz[m][hl] has k rows in parity half, 0 in other
        kz = [[qk_pool.tile([128, S], f32r, name=f"kz{m}h{hl}",
                            tag=f"kz{m}h{hl}") for hl in range(HG)]
              for m in range(2)]
        for m in range(2):
            for hl in range(HG):
                zh = 1 - (hl % 2)          # the half that must be zero
                zsl = slice(zh * 64, (zh + 1) * 64)
                nc.sync.dma_start(out=kz[m][hl][zsl, :], in_=zpad_d)

        # ---------------- projections ----------------
        projstack = ExitStack()
        projw = projstack.enter_context(tc.tile_pool(name="projw", bufs=1))
        projx = projstack.enter_context(tc.tile_pool(name="projx", bufs=1))

        wqk_sb = [projw.tile([128, D], f32r, name=f"wqk{k}", tag=f"wqk{k}")
                  for k in range(KT)]
        for k in range(KT):
            nc.sync.dma_start(out=wqk_sb[k],
                              in_=wqk_d[k * 128:(k + 1) * 128, :])
        wv_sb = [projw.tile([128, HG * DH], f32r, name=f"wv{k}", tag=f"wv{k}")
                 for k in range(KT)]
        for k in range(KT):
            nc.sync.dma_start(out=wv_sb[k],
                              in_=wv_d[k * 128:(k + 1) * 128, :])

        for nc_i in range(NCH):
            nsl = slice(nc_i * 512, (nc_i + 1) * 512)
            xtc = []
            for k in range(KT):
                x_one = projx.tile([128, 512], f32r, name="xtc", tag="xtc",
                                   bufs=8)
                nc.sync.dma_start(out=x_one,
                                  in_=xt_d[k * 128:(k + 1) * 128, nsl])
                xtc.append(x_one)
            # wqk col blocks: mt 0..3 = q1p0,q1p1,q2p0,q2p1; 4..7 = k1,k1,k2,k2
            for mt in (0, 2, 4, 6, 1, 3, 5, 7):
                pp = ps.tile([128, 512], f32, name="accp", tag="acc", bufs=4)
                for k in range(KT):
                    nc.tensor.matmul(
                        pp,
                        wqk_sb[k][:, mt * 128:(mt + 1) * 128],
                        xtc[k],
                        start=(k == 0), stop=(k == KT - 1))
                if mt < 4:
                    m, p = mt // 2, mt % 2
                    nc.vector.tensor_scalar_add(q_t[m][p][:, nsl], pp,
                                                bqk_sb[:, mt:mt + 1])
                else:
                    m, pr = (mt - 4) // 2, (mt - 4) % 2
                    for eps in range(2):
                        hl = 2 * pr + eps
                        esl = slice(eps * 64, (eps + 1) * 64)
                        nc.vector.tensor_scalar_add(
                            kz[m][hl][esl, nsl], pp[esl, :],
                            bqk_sb[esl, mt:mt + 1])
            # v projection for the 4 key tiles of this chunk
            for sl in range(4):
                st = nc_i * 4 + sl
                vp = ps.tile([128, HG * DH], f32, name="accv", tag="acc",
                             bufs=4)
                for k in range(KT):
                    nc.tensor.matmul(
                        vp,
                        xtc[k][:, sl * 128:(sl + 1) * 128],
                        wv_sb[k],
                        start=(k == 0), stop=(k == KT - 1))
                nc.vector.tensor_copy(
                    out=v_sb[:, st, :, 0:DH],
                    in_=vp.rearrange("p (h d) -> p h d", h=HG))

        projstack.close()

        # ---------------- attention ----------------
        e_pool = ctx.enter_context(tc.tile_pool(name="ep", bufs=2))
        oc_pool = ctx.enter_context(tc.tile_pool(name="oc", bufs=6))
        small = ctx.enter_context(tc.tile_pool(name="small", bufs=2))
        outst_pool = ctx.enter_context(tc.tile_pool(name="outst", bufs=3))
        scr_pool = ctx.enter_context(tc.tile_pool(name="scr", bufs=2,
                                                  space="DRAM"))

        for c in range(NCH):
            csl = slice(c * 512, (c + 1) * 512)
            ochl = [None] * HG
            for p in range(2):
                u_tiles = []
                for name in ("u1a", "u1b", "u2a", "u2b"):
                    u_tiles.append(ps.tile([DH + 1, 512], f32, name=name,
                                           tag="acc", bufs=4))
                for j in range(JT):
                    jsl = slice(j * 128, (j + 1) * 128)
                    # split score tiles (2 banks each) so next iteration's
                    # score matmuls overlap this iteration's exp -- a single
                    # 4-bank tile serializes PE behind ACT and lets the HAM
                    # clock-gate throttle the PE to 1.2 GHz
                    e_m = []
                    for m in range(2):
                        s_ps = ps.tile([128, 1024], f32, name=f"s{m}",
                                       tag=f"s{m}", bufs=1)
                        for eps in range(2):
                            nc.tensor.matmul(
                                s_ps[:, eps * 512:(eps + 1) * 512],
                                kz[m][2 * p + eps][:, jsl],
                                q_t[m][p][:, csl],
                                start=True, stop=True)
                        e_sb = e_pool.tile([128, 1024], f32r, name=f"e{m}",
                                           tag=f"e{m}")
                        nc.scalar.activation(e_sb, s_ps, Exp,
                                             bias=maskb_sb[:, j:j + 1],
                                             scale=SCALE)
                        e_m.append(e_sb)
                    # u accumulation; eps-outer so consecutive matmuls share
                    # the same stationary v tile
                    for eps in range(2):
                        for mi in range(2):
                            nc.tensor.matmul(
                                u_tiles[2 * mi + eps],
                                v_sb[:, j, 2 * p + eps, :],
                                e_m[mi][:, eps * 512:(eps + 1) * 512],
                                start=(j == 0), stop=(j == JT - 1))
                # combine: o_hl = u1/dn1 - lam*u2/dn2 (+ (1-lam)*bv)
                for eps in range(2):
                    hl = 2 * p + eps
                    u1 = u_tiles[0 + eps]
                    u2 = u_tiles[2 + eps]
                    u1_sb = small.tile([64, 512], f32, name="u1_sb",
                                       tag="u1_sb")
                    nc.vector.tensor_copy(out=u1_sb, in_=u1[0:DH, :])
                    u2_sb = small.tile([64, 512], f32, name="u2_sb",
                                       tag="u2_sb")
                    nc.vector.tensor_copy(out=u2_sb, in_=u2[0:DH, :])
                    rb = small.tile([DH + 1, 1024], f32, name="rb", tag="rb")
                    nc.vector.reciprocal(rb[DH:DH + 1, 0:512],
                                         u1[DH:DH + 1, :])
                    nc.vector.reciprocal(rb[DH:DH + 1, 512:1024],
                                         u2[DH:DH + 1, :])
                    # partition-broadcast r via a DRAM round-trip
                    scr = scr_pool.tile([1, 1024], f32, name="scr", tag="scr")
                    nc.sync.dma_start(out=scr, in_=rb[DH:DH + 1, :])
                    bc = small.tile([64, 1024], f32, name="bc", tag="bc")
                    nc.gpsimd.dma_start(
                        out=bc, in_=scr.partition_broadcast(64)[:, 0, :])
                    t1 = small.tile([64, 512], f32, name="t1", tag="t1")
                    nc.vector.tensor_tensor(t1, u1_sb, bc[:, 0:512], mult)
                    t2 = small.tile([64, 512], f32, name="t2", tag="t2")
                    nc.vector.tensor_tensor(t2, u2_sb, bc[:, 512:1024], mult)
                    oc_t = oc_pool.tile([64, 512], f32r, name="oc_t",
                                        tag="oc")
                    # oc = t1 - lam*t2 ; bv correction folded in by host when
                    # bv != 0 via bvc (adds one op)
                    nc.vector.scalar_tensor_tensor(
                        out=oc_t, in0=t2, scalar=-float(lam), in1=t1,
                        op0=mult, op1=add)
                    ochl[hl] = oc_t

            # out projection for this query chunk (K=64 per local head)
            for mt in range(MT):
                op = ps.tile([128, 512], f32, name="accop", tag="acc", bufs=4)
                for hl in range(HG):
                    nc.tensor.matmul(op,
                                     wo_sb[hl][:, mt * 128:(mt + 1) * 128],
                                     ochl[hl],
                                     start=(hl == 0), stop=(hl == HG - 1))
                outst = outst_pool.tile([128, 512], f32, name="outst",
                                        tag="outst")
                nc.vector.tensor_copy(out=outst, in_=op)
                nc.sync.dma_start(out=out_d[mt * 128:(mt + 1) * 128, csl],
                                  in_=outst)

    nc.compile()
    return nc


def _get_nc(lam: float):
    key = round(float(lam), 8)
    if key not in _BUILD_CACHE:
        _BUILD_CACHE[key] = _build(float(lam))
    return _BUILD_CACHE[key]


def _prep_in_maps(hidden_states, attention_mask, Wq, bq, Wk, bk, Wv, bv, Wo,
                  lam_f):
    in_maps = []
    for d in range(NCORES):
        b, g = d // 4, d % 4
        gc = slice(g * HG * DH, (g + 1) * HG * DH)   # 256 head-group columns
        xt = np.ascontiguousarray(hidden_states[b].T)
        wqk = np.ascontiguousarray(
            np.concatenate([Wq[:, :D][:, gc], Wq[:, D:][:, gc],
                            Wk[:, :D][:, gc], Wk[:, D:][:, gc]], axis=1))
        wv = np.ascontiguousarray(Wv[:, gc])
        wo = np.ascontiguousarray(Wo[gc, :])
        bqk = np.ascontiguousarray(
            np.concatenate([bq[:D][gc], bq[D:][gc], bk[:D][gc], bk[D:][gc]])
            .reshape(MT, 128).T)
        bvc = np.ascontiguousarray(
            ((1.0 - lam_f) * bv[gc]).reshape(HG, 64).T)
        maskb = np.ascontiguousarray(
            ((1.0 - attention_mask[b]) * -10000.0).reshape(JT, 128).T)
        in_maps.append({"xt": xt, "wqk": wqk, "wv": wv, "wo": wo,
                        "bqk": bqk, "bvc": bvc, "maskb": maskb,
                        "zpad": np.zeros((64, S), np.float32)})
    return in_maps


def kernel(hidden_states, attention_mask, Wq, bq, Wk, bk, Wv, bv, Wo, bo,
           lam):
    hidden_states = np.asarray(hidden_states, dtype=np.float32)
    attention_mask = np.asarray(attention_mask, dtype=np.float32)
    Wq = np.asarray(Wq, dtype=np.float32)
    bq = np.asarray(bq, dtype=np.float32)
    Wk = np.asarray(Wk, dtype=np.float32)
    bk = np.asarray(bk, dtype=np.float32)
    Wv = np.asarray(Wv, dtype=np.float32)
    bv = np.asarray(bv, dtype=np.float32)
    Wo = np.asarray(Wo, dtype=np.float32)
    bo = np.asarray(bo, dtype=np.float32)
    lam_f = float(np.asarray(lam))

    from concourse.bass_utils import run_bass_kernel_spmd

    nc = _get_nc(lam_f)
    in_maps = _prep_in_maps(hidden_states, attention_mask, Wq, bq, Wk, bk,
                            Wv, bv, Wo, lam_f)
    res = run_bass_kernel_spmd(nc, in_maps, core_ids=list(range(NCORES)))

    out = np.zeros((B, S, D), np.float32)
    for d in range(NCORES):
        out[d // 4] += res.results[d]["outT"].T
    out += bo
    # v-bias correction is linear: o += (1-lam)*bv @ Wo (exact; bv is zero in
    # the reference setup, so this is a no-op there)
    if np.any(bv != 0.0):
        out += ((1.0 - lam_f) * bv) @ Wo
    return out


# revision 14
# speedup vs baseline: 1.7072x; 1.1159x over previous
"""Differential attention kernel for 8 trn2 NeuronCores.

Sharding: (batch, head-group) over 8 cores. Core d handles batch b=d//4 and
heads [4*(d%4), 4*(d%4)+4). Each core:
  - projects q1,q2 in transposed pair layout qT [128, S] (2 heads per tile)
    and k1,k2 into per-head ZERO-PADDED tiles kz [128, S] (head rows in the
    matching parity half, other half zeroed) so every score matmul runs with
    K=128 -- fp32r matmuls at K=64 with alternating row groups are ~5x
    slower on TRN2, while K=128 runs at N cycles,
  - v in direct layout [S, 4, 65] with a ones column (denominator trick),
  - scores TRANSPOSED sT[j, i] (keys on partitions): mask bias becomes a
    per-partition ACT bias, PV matmul needs no transposes; exp via one
    [128, 2048] ACT op per key tile covering s1/s2 for both heads of a pair,
  - uT[65, 512] accumulated over key tiles; row 64 = softmax denominators,
  - combine o = u1/dn1 - lam*u2/dn2 via reciprocal_approx_fast + a DRAM
    round-trip partition-broadcast; lam folded into one scalar_tensor_tensor,
  - out-projection 4-way K=64 accumulation of Wo_hl.T @ o_hl -> partial
    outT [1024, S].
Host sums the 4 partial outT per batch (+bo) and transposes.
"""
import numpy as np

B, S, D, H = 2, 2048, 1024, 16
DH = D // H          # 64
SCALE = DH ** -0.5   # 0.125
NCORES = 8
HG = 4               # heads per device
KT = D // 128        # 8 contraction tiles over D
MT = D // 128        # 8 output tiles of qk projection (q1,q2,k1,k2 cols)
NCH = S // 512       # 4 query chunks
JT = S // 128        # 16 key tiles

_BUILD_CACHE = {}


def _build(lam: float):
    from contextlib import ExitStack
    import concourse.mybir as mybir
    import concourse.tile as tile
    from concourse import bacc

    f32 = mybir.dt.float32
    f32r = mybir.dt.float32r
    Exp = mybir.ActivationFunctionType.Exp
    mult = mybir.AluOpType.mult
    add = mybir.AluOpType.add

    nc = bacc.Bacc("TRN2", target_bir_lowering=False, debug=False,
                   num_devices=NCORES)

    xt_d = nc.dram_tensor("xt", [D, S], f32r, kind="ExternalInput").ap()
    wqk_d = nc.dram_tensor("wqk", [D, D], f32r, kind="ExternalInput").ap()
    wv_d = nc.dram_tensor("wv", [D, HG * DH], f32r, kind="ExternalInput").ap()
    wo_d = nc.dram_tensor("wo", [HG * DH, D], f32r, kind="ExternalInput").ap()
    bqk_d = nc.dram_tensor("bqk", [128, MT], f32, kind="ExternalInput").ap()
    bvc_d = nc.dram_tensor("bvc", [64, HG], f32, kind="ExternalInput").ap()
    maskb_d = nc.dram_tensor("maskb", [128, JT], f32,
                             kind="ExternalInput").ap()
    zpad_d = nc.dram_tensor("zpad", [64, S], f32r, kind="ExternalInput").ap()
    out_d = nc.dram_tensor("outT", [D, S], f32, kind="ExternalOutput").ap()

    with tile.TileContext(nc) as tc, ExitStack() as ctx:
        consts = ctx.enter_context(tc.tile_pool(name="consts", bufs=1))
        qk_pool = ctx.enter_context(tc.tile_pool(name="qk", bufs=1))
        v_pool = ctx.enter_context(tc.tile_pool(name="vp", bufs=1))
        ps = ctx.enter_context(tc.tile_pool(name="ps", bufs=1, space="PSUM"))

        bqk_sb = consts.tile([128, MT], f32)
        nc.sync.dma_start(out=bqk_sb, in_=bqk_d)
        bvc_sb = consts.tile([64, HG], f32)
        nc.sync.dma_start(out=bvc_sb, in_=bvc_d)
        maskb_sb = consts.tile([128, JT], f32)
        nc.sync.dma_start(out=maskb_sb, in_=maskb_d)
        # Wo rows grouped per local head hl
        wo_sb = [consts.tile([64, D], f32r, name=f"wo{i}", tag=f"wo{i}")
                 for i in range(HG)]
        for i in range(HG):
            nc.sync.dma_start(out=wo_sb[i], in_=wo_d[i * 64:(i + 1) * 64, :])
        ones1 = consts.tile([128, 1], f32)
        nc.vector.memset(ones1, 1.0)
        zeros1 = consts.tile([128, 1], f32)
        nc.vector.memset(zeros1, 0.0)

        # v in [S, HG, DH+1] layout; column DH holds ones (denominator trick)
        v_sb = v_pool.tile([128, JT, HG, DH + 1], f32r)
        nc.vector.tensor_copy(
            out=v_sb[:, :, :, DH:DH + 1],
            in_=ones1[:, None, None, :].broadcast_to([128, JT, HG, 1]))

        # q pair tiles: q_t[m][p], heads 2p (rows 0:64) and 2p+1 (rows 64:128)
        q_t = [[qk_pool.tile([128, S], f32r, name=f"q{m}p{p}",
                             tag=f"q{m}p{p}") for p in range(2)]
               for m in range(2)]
        # zero-padded k tiles: kz[m][hl] has k rows in parity half, 0 in other
        kz = [[qk_pool.tile([128, S], f32r, name=f"kz{m}h{hl}",
                            tag=f"kz{m}h{hl}") for hl in range(HG)]
              for m in range(2)]
        for m in range(2):
            for hl in range(HG):
                zh = 1 - (hl % 2)          # the half that must be zero
                zsl = slice(zh * 64, (zh + 1) * 64)
                nc.sync.dma_start(out=kz[m][hl][zsl, :], in_=zpad_d)

        # ---------------- projections ----------------
        projstack = ExitStack()
        projw = projstack.enter_context(tc.tile_pool(name="projw", bufs=1))
        projx = projstack.enter_context(tc.tile_pool(name="projx", bufs=1))

        wqk_sb = [projw.tile([128, D], f32r, name=f"wqk{k}", tag=f"wqk{k}")
                  for k in range(KT)]
        for k in range(KT):
            nc.sync.dma_start(out=wqk_sb[k],
                              in_=wqk_d[k * 128:(k + 1) * 128, :])
        wv_sb = [projw.tile([128, HG * DH], f32r, name=f"wv{k}", tag=f"wv{k}")
                 for k in range(KT)]
        for k in range(KT):
            nc.sync.dma_start(out=wv_sb[k],
                              in_=wv_d[k * 128:(k + 1) * 128, :])

        for nc_i in range(NCH):
            nsl = slice(nc_i * 512, (nc_i + 1) * 512)
            xtc = []
            for k in range(KT):
                x_one = projx.tile([128, 512], f32r, name="xtc", tag="xtc",
                                   bufs=8)
                nc.sync.dma_start(out=x_one,
                                  in_=xt_d[k * 128:(k + 1) * 128, nsl])
                xtc.append(x_one)
            # wqk col blocks: mt 0..3 = q1p0,q1p1,q2p0,q2p1; 4..7 = k1,k1,k2,k2
            # k-OUTER rounds (4 psum accumulators held per round) so the
            # first matmul only needs the first weight tile, not all of wqk;
            # round order favors pair 0 (mts 0,2,4,6), then v, then pair 1
            def qk_round(mts):
                pps = [ps.tile([128, 512], f32, name="accp", tag="acc",
                               bufs=4) for _ in mts]
                for k in range(KT):
                    for i, mt in enumerate(mts):
                        nc.tensor.matmul(
                            pps[i],
                            wqk_sb[k][:, mt * 128:(mt + 1) * 128],
                            xtc[k],
                            start=(k == 0), stop=(k == KT - 1))
                for i, mt in enumerate(mts):
                    pp = pps[i]
                    if mt < 4:
                        m, p = mt // 2, mt % 2
                        nc.vector.tensor_scalar_add(q_t[m][p][:, nsl], pp,
                                                    bqk_sb[:, mt:mt + 1])
                    else:
                        m, pr = (mt - 4) // 2, (mt - 4) % 2
                        for eps in range(2):
                            hl = 2 * pr + eps
                            esl = slice(eps * 64, (eps + 1) * 64)
                            nc.vector.tensor_scalar_add(
                                kz[m][hl][esl, nsl], pp[esl, :],
                                bqk_sb[esl, mt:mt + 1])

            qk_round((0, 2, 4, 6))
            # v projection for the 4 key tiles of this chunk, k-outer
            vps = [ps.tile([128, HG * DH], f32, name="accv", tag="acc",
                           bufs=4) for _ in range(4)]
            for k in range(KT):
                for sl in range(4):
                    nc.tensor.matmul(
                        vps[sl],
                        xtc[k][:, sl * 128:(sl + 1) * 128],
                        wv_sb[k],
                        start=(k == 0), stop=(k == KT - 1))
            for sl in range(4):
                st = nc_i * 4 + sl
                nc.vector.tensor_copy(
                    out=v_sb[:, st, :, 0:DH],
                    in_=vps[sl].rearrange("p (h d) -> p h d", h=HG))
            qk_round((1, 3, 5, 7))

        projstack.close()

        # ---------------- attention ----------------
        e_pool = ctx.enter_context(tc.tile_pool(name="ep", bufs=2))
        oc_pool = ctx.enter_context(tc.tile_pool(name="oc", bufs=6))
        small = ctx.enter_context(tc.tile_pool(name="small", bufs=2))
        outst_pool = ctx.enter_context(tc.tile_pool(name="outst", bufs=3))
        scr_pool = ctx.enter_context(tc.tile_pool(name="scr", bufs=2,
                                                  space="DRAM"))

        for c in range(NCH):
            csl = slice(c * 512, (c + 1) * 512)
            ochl = [None] * HG
            for p in range(2):
                u_tiles = []
                for name in ("u1a", "u1b", "u2a", "u2b"):
                    u_tiles.append(ps.tile([DH + 1, 512], f32, name=name,
                                           tag="acc", bufs=4))
                for j in range(JT):
                    jsl = slice(j * 128, (j + 1) * 128)
                    # split score tiles (2 banks each) so next iteration's
                    # score matmuls overlap this iteration's exp -- a single
                    # 4-bank tile serializes PE behind ACT and lets the HAM
                    # clock-gate throttle the PE to 1.2 GHz
                    e_m = []
                    for m in range(2):
                        s_ps = ps.tile([128, 1024], f32, name=f"s{m}",
                                       tag=f"s{m}", bufs=1)
                        for eps in range(2):
                            nc.tensor.matmul(
                                s_ps[:, eps * 512:(eps + 1) * 512],
                                kz[m][2 * p + eps][:, jsl],
                                q_t[m][p][:, csl],
                                start=True, stop=True)
                        e_sb = e_pool.tile([128, 1024], f32r, name=f"e{m}",
                                           tag=f"e{m}")
                        nc.scalar.activation(e_sb, s_ps, Exp,
                                             bias=maskb_sb[:, j:j + 1],
                                             scale=SCALE)
                        e_m.append(e_sb)
                    # u accumulation; eps-outer so consecutive matmuls share
                    # the same stationary v tile
                    for eps in range(2):
                        for mi in range(2):
                            nc.tensor.matmul(
                                u_tiles[2 * mi + eps],
                                v_sb[:, j, 2 * p + eps, :],
                                e_m[mi][:, eps * 512:(eps + 1) * 512],
                                start=(j == 0), stop=(j == JT - 1))
                # combine: o_hl = u1/dn1 - lam*u2/dn2.
                # reciprocal_approx_fast corrupts with a PSUM source, so
                # stage the 4 denominator rows (ACT copies, base 64), DMA
                # them to partition rows 0..3, one approx recip, then one
                # partition-broadcast DMA via DRAM.
                u_sbs = []
                g64 = small.tile([DH + 1, 2048], f32, name="g64", tag="g64", bufs=1)
                for eps in range(2):
                    u1 = u_tiles[0 + eps]
                    u2 = u_tiles[2 + eps]
                    u1_sb = small.tile([64, 512], f32, name="u1_sb",
                                       tag="u1_sb")
                    nc.vector.tensor_copy(out=u1_sb, in_=u1[0:DH, :])
                    u2_sb = small.tile([64, 512], f32, name="u2_sb",
                                       tag="u2_sb")
                    nc.vector.tensor_copy(out=u2_sb, in_=u2[0:DH, :])
                    u_sbs.append((u1_sb, u2_sb))
                    nc.scalar.copy(
                        g64[DH:DH + 1, (2 * eps) * 512:(2 * eps + 1) * 512],
                        u1[DH:DH + 1, :])
                    nc.scalar.copy(
                        g64[DH:DH + 1,
                            (2 * eps + 1) * 512:(2 * eps + 2) * 512],
                        u2[DH:DH + 1, :])
                dng = small.tile([4, 512], f32, name="dng", tag="dng")
                nc.sync.dma_start(out=dng, in_=g64[DH:DH + 1, :])
                rg = small.tile([4, 512], f32, name="rg", tag="rg")
                nc.vector.reciprocal_approx_fast(out=rg, in_=dng)
                scr = scr_pool.tile([4, 512], f32, name="scr", tag="scr")
                nc.sync.dma_start(out=scr, in_=rg)
                bc = small.tile([64, 4, 512], f32, name="bc", tag="bc", bufs=1)
                nc.gpsimd.dma_start(out=bc, in_=scr.partition_broadcast(64))
                for eps in range(2):
                    hl = 2 * p + eps
                    u1_sb, u2_sb = u_sbs[eps]
                    t1 = small.tile([64, 512], f32, name="t1", tag="t1")
                    nc.vector.tensor_tensor(t1, u1_sb, bc[:, 2 * eps, :],
                                            mult)
                    t2 = small.tile([64, 512], f32, name="t2", tag="t2")
                    nc.vector.tensor_tensor(t2, u2_sb, bc[:, 2 * eps + 1, :],
                                            mult)
                    oc_t = oc_pool.tile([64, 512], f32r, name="oc_t",
                                        tag="oc")
                    # oc = t1 - lam*t2 ; bv correction handled on host
                    nc.vector.scalar_tensor_tensor(
                        out=oc_t, in0=t2, scalar=-float(lam), in1=t1,
                        op0=mult, op1=add)
                    ochl[hl] = oc_t

            # out projection for this query chunk (K=64 per local head)
            for mt in range(MT):
                op = ps.tile([128, 512], f32, name="accop", tag="acc", bufs=4)
                for hl in range(HG):
                    nc.tensor.matmul(op,
                                     wo_sb[hl][:, mt * 128:(mt + 1) * 128],
                                     ochl[hl],
                                     start=(hl == 0), stop=(hl == HG - 1))
                outst = outst_pool.tile([128, 512], f32, name="outst",
                                        tag="outst")
                nc.vector.tensor_copy(out=outst, in_=op)
                nc.sync.dma_start(out=out_d[mt * 128:(mt + 1) * 128, csl],
                                  in_=outst)

    nc.compile()
    return nc


def _get_nc(lam: float):
    key = round(float(lam), 8)
    if key not in _BUILD_CACHE:
        _BUILD_CACHE[key] = _build(float(lam))
    return _BUILD_CACHE[key]


def _prep_in_maps(hidden_states, attention_mask, Wq, bq, Wk, bk, Wv, bv, Wo,
                  lam_f):
    in_maps = []
    for d in range(NCORES):
        b, g = d // 4, d % 4
        gc = slice(g * HG * DH, (g + 1) * HG * DH)   # 256 head-group columns
        xt = np.ascontiguousarray(hidden_states[b].T)
        wqk = np.ascontiguousarray(
            np.concatenate([Wq[:, :D][:, gc], Wq[:, D:][:, gc],
                            Wk[:, :D][:, gc], Wk[:, D:][:, gc]], axis=1))
        wv = np.ascontiguousarray(Wv[:, gc])
        wo = np.ascontiguousarray(Wo[gc, :])
        bqk = np.ascontiguousarray(
            np.concatenate([bq[:D][gc], bq[D:][gc], bk[:D][gc], bk[D:][gc]])
            .reshape(MT, 128).T)
        bvc = np.ascontiguousarray(
            ((1.0 - lam_f) * bv[gc]).reshape(HG, 64).T)
        maskb = np.ascontiguousarray(
            ((1.0 - attention_mask[b]) * -10000.0).reshape(JT, 128).T)
        in_maps.append({"xt": xt, "wqk": wqk, "wv": wv, "wo": wo,
                        "bqk": bqk, "bvc": bvc, "maskb": maskb,
                        "zpad": np.zeros((64, S), np.float32)})
    return in_maps


def kernel(hidden_states, attention_mask, Wq, bq, Wk, bk, Wv, bv, Wo, bo,
           lam):
    hidden_states = np.asarray(hidden_states, dtype=np.float32)
    attention_mask = np.asarray(attention_mask, dtype=np.float32)
    Wq = np.asarray(Wq, dtype=np.float32)
    bq = np.asarray(bq, dtype=np.float32)
    Wk = np.asarray(Wk, dtype=np.float32)
    bk = np.asarray(bk, dtype=np.float32)
    Wv = np.asarray(Wv, dtype=np.float32)
    bv = np.asarray(bv, dtype=np.float32)
    Wo = np.asarray(Wo, dtype=np.float32)
    bo = np.asarray(bo, dtype=np.float32)
    lam_f = float(np.asarray(lam))

    from concourse.bass_utils import run_bass_kernel_spmd

    nc = _get_nc(lam_f)
    in_maps = _prep_in_maps(hidden_states, attention_mask, Wq, bq, Wk, bk,
                            Wv, bv, Wo, lam_f)
    res = run_bass_kernel_spmd(nc, in_maps, core_ids=list(range(NCORES)))

    out = np.zeros((B, S, D), np.float32)
    for d in range(NCORES):
        out[d // 4] += res.results[d]["outT"].T
    out += bo
    # v-bias correction is linear: o += (1-lam)*bv @ Wo (exact; bv is zero in
    # the reference setup, so this is a no-op there)
    if np.any(bv != 0.0):
        out += ((1.0 - lam_f) * bv) @ Wo
    return out


# revision 15
# speedup vs baseline: 1.7273x; 1.0118x over previous
"""Differential attention kernel for 8 trn2 NeuronCores.

Sharding: (batch, head-group) over 8 cores. Core d handles batch b=d//4 and
heads [4*(d%4), 4*(d%4)+4). Each core:
  - projects q1,q2 in transposed pair layout qT [128, S] (2 heads per tile)
    and k1,k2 into per-head ZERO-PADDED tiles kz [128, S] (head rows in the
    matching parity half, other half zeroed) so every score matmul runs with
    K=128 -- fp32r matmuls at K=64 with alternating row groups are ~5x
    slower on TRN2, while K=128 runs at N cycles,
  - v in direct layout [S, 4, 65] with a ones column (denominator trick),
  - scores TRANSPOSED sT[j, i] (keys on partitions): mask bias becomes a
    per-partition ACT bias, PV matmul needs no transposes; exp via one
    [128, 2048] ACT op per key tile covering s1/s2 for both heads of a pair,
  - uT[65, 512] accumulated over key tiles; row 64 = softmax denominators,
  - combine o = u1/dn1 - lam*u2/dn2 via reciprocal_approx_fast + a DRAM
    round-trip partition-broadcast; lam folded into one scalar_tensor_tensor,
  - out-projection 4-way K=64 accumulation of Wo_hl.T @ o_hl -> partial
    outT [1024, S].
Host sums the 4 partial outT per batch (+bo) and transposes.
"""
import numpy as np

B, S, D, H = 2, 2048, 1024, 16
DH = D // H          # 64
SCALE = DH ** -0.5   # 0.125
NCORES = 8
HG = 4               # heads per device
KT = D // 128        # 8 contraction tiles over D
MT = D // 128        # 8 output tiles of qk projection (q1,q2,k1,k2 cols)
NCH = S // 512       # 4 query chunks
JT = S // 128        # 16 key tiles

_BUILD_CACHE = {}


def _build(lam: float):
    from contextlib import ExitStack
    import concourse.mybir as mybir
    import concourse.tile as tile
    from concourse import bacc

    f32 = mybir.dt.float32
    f32r = mybir.dt.float32r
    Exp = mybir.ActivationFunctionType.Exp
    mult = mybir.AluOpType.mult
    add = mybir.AluOpType.add

    nc = bacc.Bacc("TRN2", target_bir_lowering=False, debug=False,
                   num_devices=NCORES)

    xt_d = nc.dram_tensor("xt", [D, S], f32r, kind="ExternalInput").ap()
    wqk_d = nc.dram_tensor("wqk", [D, D], f32r, kind="ExternalInput").ap()
    wv_d = nc.dram_tensor("wv", [D, HG * DH], f32r, kind="ExternalInput").ap()
    wo_d = nc.dram_tensor("wo", [HG * DH, D], f32r, kind="ExternalInput").ap()
    bqk_d = nc.dram_tensor("bqk", [128, MT], f32, kind="ExternalInput").ap()
    bvc_d = nc.dram_tensor("bvc", [64, HG], f32, kind="ExternalInput").ap()
    maskb_d = nc.dram_tensor("maskb", [128, JT], f32,
                             kind="ExternalInput").ap()
    zpad_d = nc.dram_tensor("zpad", [64, S], f32r, kind="ExternalInput").ap()
    out_d = nc.dram_tensor("outT", [D, S], f32, kind="ExternalOutput").ap()

    with tile.TileContext(nc) as tc, ExitStack() as ctx:
        consts = ctx.enter_context(tc.tile_pool(name="consts", bufs=1))
        qk_pool = ctx.enter_context(tc.tile_pool(name="qk", bufs=1))
        v_pool = ctx.enter_context(tc.tile_pool(name="vp", bufs=1))
        ps = ctx.enter_context(tc.tile_pool(name="ps", bufs=1, space="PSUM"))

        bqk_sb = consts.tile([128, MT], f32)
        nc.sync.dma_start(out=bqk_sb, in_=bqk_d)
        bvc_sb = consts.tile([64, HG], f32)
        nc.sync.dma_start(out=bvc_sb, in_=bvc_d)
        maskb_sb = consts.tile([128, JT], f32)
        nc.sync.dma_start(out=maskb_sb, in_=maskb_d)
        # Wo rows grouped per local head hl
        wo_sb = [consts.tile([64, D], f32r, name=f"wo{i}", tag=f"wo{i}")
                 for i in range(HG)]
        for i in range(HG):
            nc.sync.dma_start(out=wo_sb[i], in_=wo_d[i * 64:(i + 1) * 64, :])
        ones1 = consts.tile([128, 1], f32)
        nc.vector.memset(ones1, 1.0)
        zeros1 = consts.tile([128, 1], f32)
        nc.vector.memset(zeros1, 0.0)

        # v in [S, HG, DH+1] layout; column DH holds ones (denominator trick)
        v_sb = v_pool.tile([128, JT, HG, DH + 1], f32r)
        nc.vector.tensor_copy(
            out=v_sb[:, :, :, DH:DH + 1],
            in_=ones1[:, None, None, :].broadcast_to([128, JT, HG, 1]))

        # q pair tiles: q_t[m][p], heads 2p (rows 0:64) and 2p+1 (rows 64:128)
        q_t = [[qk_pool.tile([128, S], f32r, name=f"q{m}p{p}",
                             tag=f"q{m}p{p}") for p in range(2)]
               for m in range(2)]
        # zero-padded k tiles: kz[m][hl] has k rows in parity half, 0 in other
        kz = [[qk_pool.tile([128, S], f32r, name=f"kz{m}h{hl}",
                            tag=f"kz{m}h{hl}") for hl in range(HG)]
              for m in range(2)]
        for m in range(2):
            for hl in range(HG):
                zh = 1 - (hl % 2)          # the half that must be zero
                zsl = slice(zh * 64, (zh + 1) * 64)
                nc.sync.dma_start(out=kz[m][hl][zsl, :], in_=zpad_d)

        # ---------------- projections ----------------
        projstack = ExitStack()
        projw = projstack.enter_context(tc.tile_pool(name="projw", bufs=1))
        projx = projstack.enter_context(tc.tile_pool(name="projx", bufs=1))

        wqk_sb = [projw.tile([128, D], f32r, name=f"wqk{k}", tag=f"wqk{k}")
                  for k in range(KT)]
        for k in range(KT):
            nc.sync.dma_start(out=wqk_sb[k],
                              in_=wqk_d[k * 128:(k + 1) * 128, :])
        wv_sb = [projw.tile([128, HG * DH], f32r, name=f"wv{k}", tag=f"wv{k}")
                 for k in range(KT)]
        for k in range(KT):
            nc.sync.dma_start(out=wv_sb[k],
                              in_=wv_d[k * 128:(k + 1) * 128, :])

        for nc_i in range(NCH):
            nsl = slice(nc_i * 512, (nc_i + 1) * 512)
            xtc = []
            for k in range(KT):
                x_one = projx.tile([128, 512], f32r, name="xtc", tag="xtc",
                                   bufs=8)
                nc.sync.dma_start(out=x_one,
                                  in_=xt_d[k * 128:(k + 1) * 128, nsl])
                xtc.append(x_one)
            # wqk col blocks: mt 0..3 = q1p0,q1p1,q2p0,q2p1; 4..7 = k1,k1,k2,k2
            # k-OUTER rounds (4 psum accumulators held per round) so the
            # first matmul only needs the first weight tile, not all of wqk;
            # round order favors pair 0 (mts 0,2,4,6), then v, then pair 1
            def qk_round(mts):
                pps = [ps.tile([128, 512], f32, name="accp", tag="acc",
                               bufs=4) for _ in mts]
                for k in range(KT):
                    for i, mt in enumerate(mts):
                        nc.tensor.matmul(
                            pps[i],
                            wqk_sb[k][:, mt * 128:(mt + 1) * 128],
                            xtc[k],
                            start=(k == 0), stop=(k == KT - 1))
                for i, mt in enumerate(mts):
                    pp = pps[i]
                    if mt < 4:
                        m, p = mt // 2, mt % 2
                        nc.vector.tensor_scalar_add(q_t[m][p][:, nsl], pp,
                                                    bqk_sb[:, mt:mt + 1])
                    else:
                        m, pr = (mt - 4) // 2, (mt - 4) % 2
                        for eps in range(2):
                            hl = 2 * pr + eps
                            esl = slice(eps * 64, (eps + 1) * 64)
                            nc.vector.tensor_scalar_add(
                                kz[m][hl][esl, nsl], pp[esl, :],
                                bqk_sb[esl, mt:mt + 1])

            qk_round((0, 2, 4, 6))
            # v projection for the 4 key tiles of this chunk, k-outer
            vps = [ps.tile([128, HG * DH], f32, name="accv", tag="acc",
                           bufs=4) for _ in range(4)]
            for k in range(KT):
                for sl in range(4):
                    nc.tensor.matmul(
                        vps[sl],
                        xtc[k][:, sl * 128:(sl + 1) * 128],
                        wv_sb[k],
                        start=(k == 0), stop=(k == KT - 1))
            for sl in range(4):
                st = nc_i * 4 + sl
                nc.vector.tensor_copy(
                    out=v_sb[:, st, :, 0:DH],
                    in_=vps[sl].rearrange("p (h d) -> p h d", h=HG))
            qk_round((1, 3, 5, 7))

        projstack.close()

        # ---------------- attention ----------------
        e_pool = ctx.enter_context(tc.tile_pool(name="ep", bufs=2))
        oc_pool = ctx.enter_context(tc.tile_pool(name="oc", bufs=6))
        small = ctx.enter_context(tc.tile_pool(name="small", bufs=2))
        outst_pool = ctx.enter_context(tc.tile_pool(name="outst", bufs=3))
        scr_pool = ctx.enter_context(tc.tile_pool(name="scr", bufs=2,
                                                  space="DRAM"))

        for c in range(NCH):
            csl = slice(c * 512, (c + 1) * 512)
            ochl = [None] * HG
            for p in range(2):
                u_tiles = []
                for name in ("u1a", "u1b", "u2a", "u2b"):
                    u_tiles.append(ps.tile([DH + 1, 512], f32, name=name,
                                           tag="acc", bufs=4))
                for j in range(JT):
                    jsl = slice(j * 128, (j + 1) * 128)
                    # split score tiles (2 banks each) so next iteration's
                    # score matmuls overlap this iteration's exp -- a single
                    # 4-bank tile serializes PE behind ACT and lets the HAM
                    # clock-gate throttle the PE to 1.2 GHz
                    e_m = []
                    for m in range(2):
                        s_ps = ps.tile([128, 1024], f32, name=f"s{m}",
                                       tag=f"s{m}", bufs=1)
                        for eps in range(2):
                            nc.tensor.matmul(
                                s_ps[:, eps * 512:(eps + 1) * 512],
                                kz[m][2 * p + eps][:, jsl],
                                q_t[m][p][:, csl],
                                start=True, stop=True)
                        e_sb = e_pool.tile([128, 1024], f32r, name=f"e{m}",
                                           tag=f"e{m}")
                        nc.scalar.activation(e_sb, s_ps, Exp,
                                             bias=maskb_sb[:, j:j + 1],
                                             scale=SCALE)
                        e_m.append(e_sb)
                    # u accumulation; eps-outer so consecutive matmuls share
                    # the same stationary v tile
                    for eps in range(2):
                        for mi in range(2):
                            nc.tensor.matmul(
                                u_tiles[2 * mi + eps],
                                v_sb[:, j, 2 * p + eps, :],
                                e_m[mi][:, eps * 512:(eps + 1) * 512],
                                start=(j == 0), stop=(j == JT - 1))
                # combine: o_hl = u1/dn1 - lam*u2/dn2.
                # reciprocal_approx_fast corrupts with a PSUM source, so
                # stage the 4 denominator rows (ACT copies, base 64), DMA
                # them to partition rows 0..3, one approx recip, then one
                # partition-broadcast DMA via DRAM.
                u_sbs = []
                g64 = small.tile([DH + 1, 2048], f32, name="g64", tag="g64", bufs=1)
                for eps in range(2):
                    u1 = u_tiles[0 + eps]
                    u2 = u_tiles[2 + eps]
                    u1_sb = small.tile([64, 512], f32, name="u1_sb",
                                       tag="u1_sb")
                    nc.vector.tensor_copy(out=u1_sb, in_=u1[0:DH, :])
                    u2_sb = small.tile([64, 512], f32, name="u2_sb",
                                       tag="u2_sb")
                    nc.vector.tensor_copy(out=u2_sb, in_=u2[0:DH, :])
                    u_sbs.append((u1_sb, u2_sb))
                    nc.vector.tensor_copy(
                        out=g64[DH:DH + 1,
                                (2 * eps) * 512:(2 * eps + 1) * 512],
                        in_=u1[DH:DH + 1, :])
                    nc.vector.tensor_copy(
                        out=g64[DH:DH + 1,
                                (2 * eps + 1) * 512:(2 * eps + 2) * 512],
                        in_=u2[DH:DH + 1, :])
                dng = small.tile([4, 512], f32, name="dng", tag="dng")
                nc.sync.dma_start(out=dng, in_=g64[DH:DH + 1, :])
                rg = small.tile([4, 512], f32, name="rg", tag="rg")
                nc.vector.reciprocal_approx_fast(out=rg, in_=dng)
                scr = scr_pool.tile([4, 512], f32, name="scr", tag="scr")
                nc.sync.dma_start(out=scr, in_=rg)
                bc = small.tile([64, 4, 512], f32, name="bc", tag="bc", bufs=1)
                nc.gpsimd.dma_start(out=bc, in_=scr.partition_broadcast(64))
                for eps in range(2):
                    hl = 2 * p + eps
                    u1_sb, u2_sb = u_sbs[eps]
                    t1 = small.tile([64, 512], f32, name="t1", tag="t1")
                    nc.vector.tensor_tensor(t1, u1_sb, bc[:, 2 * eps, :],
                                            mult)
                    t2 = small.tile([64, 512], f32, name="t2", tag="t2")
                    nc.vector.tensor_tensor(t2, u2_sb, bc[:, 2 * eps + 1, :],
                                            mult)
                    oc_t = oc_pool.tile([64, 512], f32r, name="oc_t",
                                        tag="oc")
                    # oc = t1 - lam*t2 ; bv correction handled on host
                    nc.vector.scalar_tensor_tensor(
                        out=oc_t, in0=t2, scalar=-float(lam), in1=t1,
                        op0=mult, op1=add)
                    ochl[hl] = oc_t

            # out projection for this query chunk (K=64 per local head)
            for mt in range(MT):
                op = ps.tile([128, 512], f32, name="accop", tag="acc", bufs=4)
                for hl in range(HG):
                    nc.tensor.matmul(op,
                                     wo_sb[hl][:, mt * 128:(mt + 1) * 128],
                                     ochl[hl],
                                     start=(hl == 0), stop=(hl == HG - 1))
                outst = outst_pool.tile([128, 512], f32, name="outst",
                                        tag="outst")
                nc.vector.tensor_copy(out=outst, in_=op)
                nc.sync.dma_start(out=out_d[mt * 128:(mt + 1) * 128, csl],
                                  in_=outst)

    nc.compile()
    return nc


def _get_nc(lam: float):
    key = round(float(lam), 8)
    if key not in _BUILD_CACHE:
        _BUILD_CACHE[key] = _build(float(lam))
    return _BUILD_CACHE[key]


def _prep_in_maps(hidden_states, attention_mask, Wq, bq, Wk, bk, Wv, bv, Wo,
                  lam_f):
    in_maps = []
    for d in range(NCORES):
        b, g = d // 4, d % 4
        gc = slice(g * HG * DH, (g + 1) * HG * DH)   # 256 head-group columns
        xt = np.ascontiguousarray(hidden_states[b].T)
        wqk = np.ascontiguousarray(
            np.concatenate([Wq[:, :D][:, gc], Wq[:, D:][:, gc],
                            Wk[:, :D][:, gc], Wk[:, D:][:, gc]], axis=1))
        wv = np.ascontiguousarray(Wv[:, gc])
        wo = np.ascontiguousarray(Wo[gc, :])
        bqk = np.ascontiguousarray(
            np.concatenate([bq[:D][gc], bq[D:][gc], bk[:D][gc], bk[D:][gc]])
            .reshape(MT, 128).T)
        bvc = np.ascontiguousarray(
            ((1.0 - lam_f) * bv[gc]).reshape(HG, 64).T)
        maskb = np.ascontiguousarray(
            ((1.0 - attention_mask[b]) * -10000.0).reshape(JT, 128).T)
        in_maps.append({"xt": xt, "wqk": wqk, "wv": wv, "wo": wo,
                        "bqk": bqk, "bvc": bvc, "maskb": maskb,
                        "zpad": np.zeros((64, S), np.float32)})
    return in_maps


def kernel(hidden_states, attention_mask, Wq, bq, Wk, bk, Wv, bv, Wo, bo,
           lam):
    hidden_states = np.asarray(hidden_states, dtype=np.float32)
    attention_mask = np.asarray(attention_mask, dtype=np.float32)
    Wq = np.asarray(Wq, dtype=np.float32)
    bq = np.asarray(bq, dtype=np.float32)
    Wk = np.asarray(Wk, dtype=np.float32)
    bk = np.asarray(bk, dtype=np.float32)
    Wv = np.asarray(Wv, dtype=np.float32)
    bv = np.asarray(bv, dtype=np.float32)
    Wo = np.asarray(Wo, dtype=np.float32)
    bo = np.asarray(bo, dtype=np.float32)
    lam_f = float(np.asarray(lam))

    from concourse.bass_utils import run_bass_kernel_spmd

    nc = _get_nc(lam_f)
    in_maps = _prep_in_maps(hidden_states, attention_mask, Wq, bq, Wk, bk,
                            Wv, bv, Wo, lam_f)
    res = run_bass_kernel_spmd(nc, in_maps, core_ids=list(range(NCORES)))

    out = np.zeros((B, S, D), np.float32)
    for d in range(NCORES):
        out[d // 4] += res.results[d]["outT"].T
    out += bo
    # v-bias correction is linear: o += (1-lam)*bv @ Wo (exact; bv is zero in
    # the reference setup, so this is a no-op there)
    if np.any(bv != 0.0):
        out += ((1.0 - lam_f) * bv) @ Wo
    return out
